# revision 3
# baseline (speedup 1.0000x reference)
"""Multi-head causal attention (B=4, S=2048, D=1024, H=16) on 8 TRN2 NeuronCores.

Sharding: core c -> (batch b = c//2, head-group g = c%2); 8 heads per core.

Rewrite of the baseline:
 - all matmul operands bf16 (fp32 psum accumulation); host pre-casts inputs
 - paired-head scores psum tiles ([128 keys, 1024] = heads a|b across two
   psum banks) so one exp instruction covers both heads of a pair
 - causally-trimmed scores AND attn@V matmuls (column-subrange psum
   accumulation via skip_group_check; no zero-fill, no wasted columns)
 - triangular boundary mask added on GpSimd (keeps DVE off ACT's critical
   path); softmax denominator via an interleaved ones-column in v
 - fully fused schedule: no separate projection phase. Only s-block 0 of the
   q/k/v projection runs up front; later s-blocks, the output projection of
   the previous window, and the NEXT rep's input DMA triggers are all
   interleaved into the attention jt-step stream (one pending group per few
   steps), so TensorE and ActE both stay busy and rep boundaries pipeline.
"""

import numpy as np
from contextlib import ExitStack

B, S, D, H = 4, 2048, 1024, 16
HD = D // H          # 64
HPC = H // 2         # 8 heads per core
DV = HPC * HD        # 512 v-dims per core
N_CORES = 8
SB = 512             # i-tile width (matmul N)
NSB = S // SB        # 4
NJT = S // 128       # 16 j-tiles

_CACHE = {}


def _build_module():
    import os
    KREP = int(os.environ.get("KREP", "1"))
    SPSB = int(os.environ.get("SPSB", "2"))   # paired scores psum tiles (2 banks each)
    APSB = int(os.environ.get("APSB", "2"))   # attn@V accumulators (1 bank each)
    YPB = int(os.environ.get("YPB", "2"))     # projection psum groups (1 bank each)
    EPB = int(os.environ.get("EPB", "6"))     # exp sbuf tiles
    NPB = int(os.environ.get("NPB", "8"))
    LAG = int(os.environ.get("LAG", "2"))     # attn@V lags scores by LAG j-tiles
    import concourse.bacc as bacc
    import concourse.mybir as mybir
    import concourse.tile as tile
    from concourse._compat import get_trn_type

    F32 = mybir.dt.float32
    BF16 = mybir.dt.bfloat16
    EXP = mybir.ActivationFunctionType.Exp

    nc = bacc.Bacc(get_trn_type() or "TRN2", target_bir_lowering=False, debug=False)

    # ---- DRAM parameters (per core) ----
    xT = nc.declare_dram_parameter("xT", [D, S], BF16, isOutput=False)        # x[b].T
    wq = nc.declare_dram_parameter("wq", [D, DV], BF16, isOutput=False)       # (W_q,g / 8).T
    wk = nc.declare_dram_parameter("wk", [D, DV], BF16, isOutput=False)       # W_k,g.T
    wv = nc.declare_dram_parameter("wv", [D, DV], BF16, isOutput=False)       # W_v,g.T
    ow = nc.declare_dram_parameter("ow", [DV, D], BF16, isOutput=False)       # W_out[:, g].T
    bq = nc.declare_dram_parameter("bq", [DV, 1], F32, isOutput=False)        # q bias / 8
    bk = nc.declare_dram_parameter("bk", [DV, 1], F32, isOutput=False)
    kb = nc.declare_dram_parameter("kb", [S, 1], F32, isOutput=False)         # key-mask bias
    y = nc.declare_dram_parameter("y", [S, D], F32, isOutput=True)            # partial output

    with tile.TileContext(nc) as tc, ExitStack() as octx:
        # ---- pools (all rep-stable; tags rotate buffers) ----
        pers = octx.enter_context(tc.tile_pool(name="pers", bufs=1))
        wpool = octx.enter_context(tc.tile_pool(name="wpool", bufs=1))
        owpool = octx.enter_context(tc.tile_pool(name="owpool", bufs=2))
        sps = octx.enter_context(tc.tile_pool(name="sps", bufs=SPSB, space="PSUM"))
        epool = octx.enter_context(tc.tile_pool(name="epool", bufs=EPB))
        npool = octx.enter_context(tc.tile_pool(name="npool", bufs=NPB))
        ypool = octx.enter_context(tc.tile_pool(name="ypool", bufs=4))

        qT = [pers.tile([128, S], BF16, tag=f"qT{p}", name=f"qT{p}") for p in range(4)]
        kT = [pers.tile([128, S], BF16, tag=f"kT{p}", name=f"kT{p}") for p in range(4)]
        vx = [pers.tile([128, HPC * 65], BF16, tag=f"vx{j}", name=f"vx{j}") for j in range(NJT)]
        anT = [pers.tile([128, S], BF16, tag=f"anT{p}", name=f"anT{p}") for p in range(4)]
        bq_t = pers.tile([128, 4], F32, tag="bq")
        bk_t = pers.tile([128, 4], F32, tag="bk")
        kb_t = pers.tile([128, NJT], F32, tag="kb")

        nc.sync.dma_start(bq_t[:], bq[:].squeeze(1).rearrange("(t p) -> p t", p=128))
        nc.sync.dma_start(bk_t[:], bk[:].squeeze(1).rearrange("(t p) -> p t", p=128))
        nc.sync.dma_start(kb_t[:], kb[:].squeeze(1).rearrange("(t p) -> p t", p=128))

        # ones columns of vx tiles (col 64 of each 65-wide head slot)
        for j in range(NJT):
            ones_view = vx[j][:].rearrange("p (h c) -> p h c", c=65)[:, :, 64:65]
            nc.vector.memset(ones_view, 1.0)

        def alloc_inputs():
            t = {
                "wq": [wpool.tile([128, DV], BF16, tag=f"wq{d}", name=f"wq{d}")
                       for d in range(8)],
                "wk": [wpool.tile([128, DV], BF16, tag=f"wk{d}", name=f"wk{d}")
                       for d in range(8)],
                "wv": [wpool.tile([128, DV], BF16, tag=f"wv{d}", name=f"wv{d}")
                       for d in range(8)],
                "xt": [[wpool.tile([128, SB], BF16, tag=f"xt{s}_{d}", name=f"xt{s}_{d}")
                        for d in range(8)] for s in range(NSB)],
                "ow": [owpool.tile([128, SB], BF16, tag=f"ow{i}", name=f"ow{i}")
                       for i in range(8)],
            }
            return t

        def input_dma_thunks(t):
            """One thunk per pair of input-tile DMAs, in need-first order."""
            thunks = []

            def pair(f1, f2=None):
                thunks.append(lambda: (f1(), f2() if f2 else None))

            for d in range(8):
                pair(lambda d=d: nc.sync.dma_start(t["wq"][d][:], wq[128 * d:128 * d + 128, :]),
                     lambda d=d: nc.sync.dma_start(t["xt"][0][d][:], xT[128 * d:128 * d + 128, 0:SB]))
            for d in range(8):
                pair(lambda d=d: nc.sync.dma_start(t["wk"][d][:], wk[128 * d:128 * d + 128, :]),
                     lambda d=d: nc.sync.dma_start(t["wv"][d][:], wv[128 * d:128 * d + 128, :]))
            for s in range(1, NSB):
                for d in range(0, 8, 2):
                    pair(lambda s=s, d=d: nc.sync.dma_start(
                             t["xt"][s][d][:], xT[128 * d:128 * d + 128, SB * s:SB * s + SB]),
                         lambda s=s, d=d: nc.sync.dma_start(
                             t["xt"][s][d + 1][:], xT[128 * (d + 1):128 * (d + 1) + 128, SB * s:SB * s + SB]))
            for p in range(4):
                pair(lambda p=p: nc.sync.dma_start(
                         t["ow"][2 * p][:], ow[128 * p:128 * p + 128, 0:SB]),
                     lambda p=p: nc.sync.dma_start(
                         t["ow"][2 * p + 1][:], ow[128 * p:128 * p + 128, SB:2 * SB]))
            return thunks

        cur = alloc_inputs()
        for g in input_dma_thunks(cur):
            g()
        carry_tail = []

        for _rep in range(KREP):
            tiles = cur
            nxt = alloc_inputs() if _rep + 1 < KREP else None
            nxt_dma = input_dma_thunks(nxt) if nxt else []

            def qk_group(which, sblk, o, t=tiles):
                # q or k projection for one 128-row output slice of one s-block
                wt, bt, dst = ((t["wq"], bq_t, qT) if which == "q" else (t["wk"], bk_t, kT))
                ssl = slice(SB * sblk, SB * sblk + SB)
                osl = slice(128 * o, 128 * o + 128)
                ps = sps.tile([128, SB], F32, tag="yp", bufs=YPB)
                for d in range(8):
                    nc.tensor.matmul(ps[:], wt[d][:, osl], t["xt"][sblk][d][:],
                                     start=(d == 0), stop=(d == 7))
                nc.vector.tensor_scalar_add(dst[o][:, ssl], ps[:], bt[:, o:o + 1])

            def v_group(jt, t=tiles):
                sblk, ssub = divmod(jt, 4)
                ps = sps.tile([128, SB], F32, tag="yp", bufs=YPB)
                for d in range(8):
                    nc.tensor.matmul(ps[:], t["xt"][sblk][d][:, 128 * ssub:128 * ssub + 128],
                                     t["wv"][d][:], start=(d == 0), stop=(d == 7))
                dst = vx[jt][:].rearrange("p (h c) -> p h c", c=65)[:, :, 0:64]
                src = ps[:].rearrange("p (h c) -> p h c", c=64)
                nc.vector.tensor_copy(dst, src)

            def proj_group(st, ot, t=tiles):
                # one output-projection psum group: y[128 s-rows, 512 cols]
                ssl = slice(128 * st, 128 * st + 128)
                ps = sps.tile([128, SB], F32, tag="yp", bufs=YPB)
                for p in range(4):
                    nc.tensor.matmul(ps[:], anT[p][:, ssl], t["ow"][2 * p + ot][:],
                                     start=(p == 0), stop=(p == 3))
                yt = ypool.tile([128, SB], F32, tag="yt")
                nc.vector.tensor_copy(yt[:], ps[:])
                nc.sync.dma_start(y[ssl, SB * ot:SB * ot + SB], yt[:])

            def sblk_groups(s):
                # k first (needed at the next window's first scores), then
                # q, then v (only needed once attn@V reaches that j-tile)
                return ([(lambda o=o: qk_group("k", s, o)) for o in range(4)]
                        + [(lambda o=o: qk_group("q", s, o)) for o in range(4)]
                        + [(lambda j=j: v_group(j)) for j in range(4 * s, 4 * s + 4)])

            # lead-in: s-block 0 projections emitted directly, then the
            # previous rep's final-window output projection (its anT inputs
            # are long since ready, so it never stalls the PE queue head)
            for g in sblk_groups(0):
                g()
            for g in carry_tail:
                g()
            carry_tail = []

            pending = sblk_groups(1)   # emitted during window 0

            for it in range(NSB):
                i0 = SB * it
                njt = 4 * it + 4

                # stage closures per (p, jt); pipelined with attn@V LAG
                # j-tiles behind scores+exp.
                def score_stage(p, jt, state):
                    jsl = slice(128 * jt, 128 * jt + 128)
                    r = jt - 4 * it          # >=0: diagonal j-tile
                    c0 = 128 * r if r > 0 else 0  # first valid col in i-window
                    sc = sps.tile([128, 2 * SB], F32, tag="sc")
                    nc.tensor.matmul(sc[:, c0:SB], kT[p][0:64, jsl],
                                     qT[p][0:64, i0 + c0:i0 + SB],
                                     start=True, stop=True)
                    nc.tensor.matmul(sc[:, SB + c0:2 * SB], kT[p][64:128, jsl],
                                     qT[p][64:128, i0 + c0:i0 + SB],
                                     start=True, stop=True)
                    scv = sc[:].rearrange("p (h c) -> p h c", c=SB)
                    e = epool.tile([128, 2 * SB], BF16, tag="e")
                    ev = e[:].rearrange("p (h c) -> p h c", c=SB)
                    nc.scalar.activation(ev[:, :, c0:SB], scv[:, :, c0:SB], EXP,
                                         bias=kb_t[:, jt:jt + 1])
                    if r >= 0:
                        # causal boundary block at local cols c0:c0+128: zero
                        # exp entries where key offset pj exceeds query col
                        # (keep iff c_local - pj >= 0), both heads in one op
                        nc.gpsimd.affine_select(
                            out=ev[:, :, c0:c0 + 128], in_=ev[:, :, c0:c0 + 128],
                            compare_op=mybir.AluOpType.is_ge, fill=0.0,
                            base=0, pattern=[[0, 2], [1, 128]], channel_multiplier=-1,
                        )
                    state[jt] = (e, c0)

                def attnv_stage(p, jt, state, pa, pb):
                    e, c0 = state.pop(jt)
                    ev = e[:].rearrange("p (h c) -> p h c", c=SB)
                    va = vx[jt][:, 65 * (2 * p):65 * (2 * p) + 65]
                    vb = vx[jt][:, 65 * (2 * p + 1):65 * (2 * p + 1) + 65]
                    nc.tensor.matmul(pa[:, c0:SB], va, ev[:, 0, c0:SB],
                                     start=(jt == 0), stop=(jt == njt - 1),
                                     skip_group_check=True)
                    nc.tensor.matmul(pb[:, c0:SB], vb, ev[:, 1, c0:SB],
                                     start=(jt == 0), stop=(jt == njt - 1),
                                     skip_group_check=True)

                # spread the pending groups over this window's 4*njt steps
                stride = max(1, (4 * njt) // max(1, len(pending)))
                step = 0
                for p in range(4):
                    pa = sps.tile([65, SB], F32, tag="aps", bufs=APSB)
                    pb = sps.tile([65, SB], F32, tag="aps", bufs=APSB)
                    state = {}
                    for jt in range(njt + LAG):
                        if jt < njt:
                            score_stage(p, jt, state)
                        if jt >= LAG:
                            attnv_stage(p, jt - LAG, state, pa, pb)
                            step += 1
                            if step % stride == 0 and pending:
                                pending.pop(0)()
                    # free psum promptly: raw attn sums (incl. denominator
                    # row 64) to sbuf in bf16, normalize from there
                    sraw = [npool.tile([65, SB], BF16, tag=f"sraw{h}", name=f"sraw{h}")
                            for h in (0, 1)]
                    nc.vector.tensor_copy(sraw[0][:], pa[:])
                    nc.vector.tensor_copy(sraw[1][:], pb[:])
                    for half in (0, 1):
                        rec = npool.tile([1, SB], BF16, tag="rec")
                        with nc.allow_low_precision(reason="bf16 softmax denom"):
                            nc.vector.reciprocal(rec[:], sraw[half][64:65, :])
                        rb = npool.tile([64, SB], BF16, tag="rb")
                        nc.gpsimd.partition_broadcast(rb[:], rec[:])
                        out = anT[p][64 * half:64 * half + 64, i0:i0 + SB]
                        nc.vector.tensor_mul(out, sraw[half][0:64, :], rb[:])
                for g in pending:   # leftovers (shouldn't happen)
                    g()
                # next window's pending: projections two windows ahead, the
                # output projection of this window, and (late windows) the
                # next rep's input DMA triggers
                pending = (sblk_groups(it + 2) if it + 2 < NSB else []) + [
                    (lambda st=st, ot=ot: proj_group(st, ot))
                    for st in range(4 * it, 4 * it + 4) for ot in range(2)
                ]
                if it == 1:
                    pending += nxt_dma[:20]
                elif it == 2:
                    pending += nxt_dma[20:]
            # tail: last window's projection — deferred into the next
            # rep's lead-in when one exists
            if nxt is not None:
                carry_tail = pending
            else:
                for g in pending:
                    g()
            cur = nxt

    nc.compile()
    return nc


def _get_module():
    if "nc" not in _CACHE:
        _CACHE["nc"] = _build_module()
    return _CACHE["nc"]


def _host_prep(x, mask, qkv_w, qkv_b, out_w):
    """Per-core input maps."""
    import ml_dtypes
    bf16 = ml_dtypes.bfloat16
    scale = np.float32(1.0 / np.sqrt(HD))
    in_maps = []
    for c in range(N_CORES):
        b, g = divmod(c, 2)
        qr = slice(g * DV, g * DV + DV)
        kr = slice(D + g * DV, D + g * DV + DV)
        vr = slice(2 * D + g * DV, 2 * D + g * DV + DV)
        in_maps.append({
            "xT": np.ascontiguousarray(x[b].T).astype(bf16),
            "wq": np.ascontiguousarray(qkv_w[qr].T * scale).astype(bf16),
            "wk": np.ascontiguousarray(qkv_w[kr].T).astype(bf16),
            "wv": np.ascontiguousarray(qkv_w[vr].T).astype(bf16),
            "ow": np.ascontiguousarray(out_w[:, g * DV:g * DV + DV].T).astype(bf16),
            "bq": (qkv_b[qr] * scale).reshape(DV, 1).astype(np.float32),
            "bk": qkv_b[kr].reshape(DV, 1).astype(np.float32),
            "kb": np.where(mask[b] != 0, 0.0, -1e30).astype(np.float32).reshape(S, 1),
        })
    return in_maps


def _host_gather(results, qkv_b, out_b, out_w):
    # constant bias: out_b + W_out @ v_bias (v bias commutes through attention)
    bias = out_b + out_w @ qkv_b[2 * D:3 * D]
    y = np.empty((B, S, D), dtype=np.float32)
    for b in range(B):
        y[b] = results[2 * b]["y"] + results[2 * b + 1]["y"] + bias[None, :]
    return y


def kernel(x, mask, qkv_w, qkv_b, out_w, out_b):
    import time
    from concourse.bass_utils import run_bass_kernel_spmd

    nc = _get_module()
    in_maps = _host_prep(x, mask, qkv_w, qkv_b, out_w)
    last = None
    for attempt in range(3):
        try:
            res = run_bass_kernel_spmd(nc, in_maps, core_ids=list(range(N_CORES)))
            return _host_gather(res.results, qkv_b, out_b, out_w)
        except Exception as e:  # rare transient device faults: retry after recovery
            last = e
            time.sleep(10 * (attempt + 1))
    raise last
